# revision 13
# baseline (speedup 1.0000x reference)
"""CosFace margin loss kernel for Trainium2 (8 NeuronCores, batch-sharded).

out[b, c] = S * logits[b, c] - (S*M if c == labels[b] and labels[b] != -1 else 0)

Strategy: shard the 4096-row batch across 8 cores (512 rows each). Each
core streams its [512, 50257] f32 shard through SBUF in large chunks,
scaling by S (DMA-bound), then applies the per-row margin with a single
indirect scatter-add DMA of -S*M at flat positions r*C + label[r].
"""

import sys

if "/opt/trn_rl_repo" not in sys.path:
    sys.path.insert(0, "/opt/trn_rl_repo")

import numpy as np

S = 64.0
M = 0.35
BATCH = 4096
COLS = 50257
N_CORES = 8
ROWS = BATCH // N_CORES  # 512 rows per core
P = 128  # SBUF partitions
RPP = ROWS // P  # 4 rows per partition
FREE = RPP * COLS  # 201028 contiguous elements per partition
CHUNK = 12800  # free-dim tile width (51.2KB/partition per buf)
BUFS = 3

TRACE = False  # test.py sets True to capture an NTFF profile
LAST_RESULTS = None  # BassKernelResults of the most recent run (for test.py)

_nc_cache = None


def _build():
    global _nc_cache
    if _nc_cache is not None:
        return _nc_cache

    import concourse.bass as bass
    import concourse.mybir as mybir
    from concourse import bacc
    from concourse.tile import TileContext

    nc = bacc.Bacc("TRN2", target_bir_lowering=False, debug=False, num_devices=N_CORES)

    x = nc.dram_tensor("logits", [ROWS, COLS], mybir.dt.float32, kind="ExternalInput")
    fi = nc.dram_tensor("fix_idx", [P, RPP], mybir.dt.int32, kind="ExternalInput")
    y = nc.dram_tensor("out", [ROWS, COLS], mybir.dt.float32, kind="ExternalOutput")
    yfix = nc.dram_tensor("fix_out", [P, RPP], mybir.dt.float32, kind="ExternalOutput")

    # Rows 4p..4p+3 are contiguous in DRAM, so partition p gets one
    # contiguous 201028-element stripe: big, clean DMA descriptors.
    xv = x[:].rearrange("(p r) c -> p (r c)", p=P)
    yv = y[:].rearrange("(p r) c -> p (r c)", p=P)
    # [N, 1] flat view (DMA APs must be 2-D); gather coef = 1 element.
    x_flat = x[:].rearrange("a (b one) -> (a b) one", one=1)

    with TileContext(nc) as tc:
        with (
            tc.tile_pool(name="io", bufs=BUFS) as pool,
            tc.tile_pool(name="fix", bufs=1) as fpool,
        ):
            # Margin fixup interleaved into the per-engine streams so it
            # hides completely inside the main pipeline: the idx load goes
            # FIRST on the Sync ring (a tail position would queue it behind
            # every main load), the indirect gathers run on the otherwise
            # idle GpSimd at t~0, and the tiny DVE op / ACT store slot in
            # after a few chunks, by which point their inputs are long done.
            idx_t = fpool.tile([P, RPP], mybir.dt.int32)
            g_t = fpool.tile([P, RPP], mybir.dt.float32)
            nc.sync.dma_start(out=idx_t[:], in_=fi[:])
            # Gather x.flat[idx[p,j]]: HW consumes ONE offset per partition,
            # so one [128,1] gather per fix column.
            for j in range(RPP):
                nc.gpsimd.indirect_dma_start(
                    out=g_t[:, j : j + 1],
                    out_offset=None,
                    in_=x_flat,
                    in_offset=bass.IndirectOffsetOnAxis(
                        ap=idx_t[:, j : j + 1], axis=0
                    ),
                )

            # Single-ring staggered pipeline: every transfer goes through the
            # SP HWDGE ring in the order L0, L1, [S0, L2], [S1, L3], ... so
            # the SDMA engines always have a queued transfer and the muls
            # hide under the neighbors' DMA. Two separate load/store rings
            # phase-lock stochastically into a convoy (~+15% wall time).
            starts = list(range(0, FREE, CHUNK))
            n_chunks = len(starts)
            tiles = [None] * n_chunks

            def load(i):
                c0 = starts[i]
                w = min(CHUNK, FREE - c0)
                t = pool.tile([P, CHUNK], mybir.dt.float32, tag="io")
                nc.sync.dma_start(out=t[:, :w], in_=xv[:, c0 : c0 + w])
                tiles[i] = t

            load(0)
            load(1)
            for i, c0 in enumerate(starts):
                w = min(CHUNK, FREE - c0)
                t = tiles[i]
                nc.vector.tensor_scalar_mul(t[:, :w], t[:, :w], S)
                if i == 3:
                    # fix_out = (gathered - M) * S
                    nc.vector.tensor_scalar(
                        g_t[:],
                        g_t[:],
                        -M,
                        S,
                        mybir.AluOpType.add,
                        mybir.AluOpType.mult,
                    )
                nc.sync.dma_start(out=yv[:, c0 : c0 + w], in_=t[:, :w])
                if i == 4:
                    nc.sync.dma_start(out=yfix[:], in_=g_t[:])
                if i + 2 < n_chunks:
                    load(i + 2)

    nc.compile()
    _nc_cache = nc
    return _nc_cache


def _fix_arrays(labels):
    """Per-row flat gather index ([P, RPP]-ravel order: row = p*RPP + j) and
    the validity mask for the host-side merge."""
    labels = np.asarray(labels).astype(np.int64).reshape(-1)
    valid = labels != -1
    safe = np.clip(labels, 0, COLS - 1)
    rows = np.arange(labels.shape[0], dtype=np.int64)
    flat_idx = (rows * COLS + safe).astype(np.int32)
    return flat_idx, safe, valid


def kernel(**inputs):
    logits = np.ascontiguousarray(np.asarray(inputs["logits"], dtype=np.float32))
    labels = np.asarray(inputs["labels"]).reshape(-1)
    assert logits.shape == (BATCH, COLS), logits.shape
    assert labels.shape == (BATCH,), labels.shape

    from concourse.bass_utils import run_bass_kernel_spmd

    nc = _build()

    in_maps = []
    fix = []
    for c in range(N_CORES):
        r0 = c * ROWS
        flat_idx, safe, valid = _fix_arrays(labels[r0 : r0 + ROWS])
        fix.append((safe, valid))
        in_maps.append(
            {
                "logits": logits[r0 : r0 + ROWS],
                "fix_idx": flat_idx.reshape(P, RPP),
            }
        )

    global LAST_RESULTS
    LAST_RESULTS = run_bass_kernel_spmd(
        nc, in_maps, core_ids=list(range(N_CORES)), trace=TRACE
    )
    out = np.concatenate([r["out"] for r in LAST_RESULTS.results], axis=0)
    # Merge the device-computed (logit - M) * S values at each row's label.
    for c in range(N_CORES):
        safe, valid = fix[c]
        fixed = LAST_RESULTS.results[c]["fix_out"].reshape(-1)  # row p*RPP+j
        rows = np.nonzero(valid)[0]
        out[c * ROWS + rows, safe[rows]] = fixed[rows]
    return out


# revision 16
# speedup vs baseline: 1.2376x; 1.2376x over previous
"""CosFace margin loss kernel for Trainium2 (8 NeuronCores, batch-sharded).

out[b, c] = S * logits[b, c] - (S*M if c == labels[b] and labels[b] != -1 else 0)

Strategy: shard the 4096-row batch across 8 cores (512 rows each). Each
core streams its [512, 50257] f32 shard through SBUF in large chunks,
scaling by S (DMA-bound), then applies the per-row margin with a single
indirect scatter-add DMA of -S*M at flat positions r*C + label[r].
"""

import sys

if "/opt/trn_rl_repo" not in sys.path:
    sys.path.insert(0, "/opt/trn_rl_repo")

import numpy as np

S = 64.0
M = 0.35
BATCH = 4096
COLS = 50257
N_CORES = 8
ROWS = BATCH // N_CORES  # 512 rows per core
P = 128  # SBUF partitions
RPP = ROWS // P  # 4 rows per partition
FREE = RPP * COLS  # 201028 contiguous elements per partition
CHUNK = 6976  # free-dim tile width (27.25KB/partition per buf)
BUFS = 3  # per pool; separate in/out pools

TRACE = False  # test.py sets True to capture an NTFF profile
LAST_RESULTS = None  # BassKernelResults of the most recent run (for test.py)

_nc_cache = None


def _build():
    global _nc_cache
    if _nc_cache is not None:
        return _nc_cache

    import concourse.bass as bass
    import concourse.mybir as mybir
    from concourse import bacc
    from concourse.tile import TileContext

    nc = bacc.Bacc("TRN2", target_bir_lowering=False, debug=False, num_devices=N_CORES)

    x = nc.dram_tensor("logits", [ROWS, COLS], mybir.dt.float32, kind="ExternalInput")
    fi = nc.dram_tensor("fix_idx", [P, RPP], mybir.dt.int32, kind="ExternalInput")
    y = nc.dram_tensor("out", [ROWS, COLS], mybir.dt.float32, kind="ExternalOutput")
    yfix = nc.dram_tensor("fix_out", [P, RPP], mybir.dt.float32, kind="ExternalOutput")

    # Rows 4p..4p+3 are contiguous in DRAM, so partition p gets one
    # contiguous 201028-element stripe: big, clean DMA descriptors.
    xv = x[:].rearrange("(p r) c -> p (r c)", p=P)
    yv = y[:].rearrange("(p r) c -> p (r c)", p=P)
    # [N, 1] flat view (DMA APs must be 2-D); gather coef = 1 element.
    x_flat = x[:].rearrange("a (b one) -> (a b) one", one=1)

    with TileContext(nc) as tc:
        with (
            tc.tile_pool(name="pin", bufs=BUFS) as pool_in,
            tc.tile_pool(name="pout", bufs=BUFS) as pool_out,
            tc.tile_pool(name="fix", bufs=1) as fpool,
        ):
            # Margin fixup interleaved into the per-engine streams so it
            # hides completely inside the main pipeline: the idx load goes
            # FIRST on the Sync ring (a tail position would queue it behind
            # every main load), the indirect gathers run on the otherwise
            # idle GpSimd at t~0, and the tiny DVE op / ACT store slot in
            # after a few chunks, by which point their inputs are long done.
            idx_t = fpool.tile([P, RPP], mybir.dt.int32)
            g_t = fpool.tile([P, RPP], mybir.dt.float32)
            nc.sync.dma_start(out=idx_t[:], in_=fi[:])
            # Gather x.flat[idx[p,j]]: HW consumes ONE offset per partition,
            # so one [128,1] gather per fix column.
            for j in range(RPP):
                nc.gpsimd.indirect_dma_start(
                    out=g_t[:, j : j + 1],
                    out_offset=None,
                    in_=x_flat,
                    in_offset=bass.IndirectOffsetOnAxis(
                        ap=idx_t[:, j : j + 1], axis=0
                    ),
                )

            # Separate in/out tiles: loads WAR-depend only on muls (cheap,
            # plentiful) and stores only RAW-depend on muls — never DMA on
            # DMA. With a shared in-place tile, each load waits on a store
            # COMPLETION to reuse the slot, and the load->mul->store->load
            # loop goes latency-bound (~37us/chunk); worse, it serializes
            # load and store traffic in time, so HBM runs unidirectional
            # (~341 GB/s) instead of bidirectional (~425 GB/s).
            for i, c0 in enumerate(range(0, FREE, CHUNK)):
                w = min(CHUNK, FREE - c0)
                ti = pool_in.tile([P, CHUNK], mybir.dt.float32)
                to = pool_out.tile([P, CHUNK], mybir.dt.float32)
                nc.sync.dma_start(out=ti[:, :w], in_=xv[:, c0 : c0 + w])
                nc.vector.tensor_scalar_mul(to[:, :w], ti[:, :w], S)
                if i == 3:
                    # fix_out = (gathered - M) * S
                    nc.vector.tensor_scalar(
                        g_t[:],
                        g_t[:],
                        -M,
                        S,
                        mybir.AluOpType.add,
                        mybir.AluOpType.mult,
                    )
                nc.scalar.dma_start(out=yv[:, c0 : c0 + w], in_=to[:, :w])
                if i == 4:
                    nc.scalar.dma_start(out=yfix[:], in_=g_t[:])

    nc.compile()
    _nc_cache = nc
    return _nc_cache


def _fix_arrays(labels):
    """Per-row flat gather index ([P, RPP]-ravel order: row = p*RPP + j) and
    the validity mask for the host-side merge."""
    labels = np.asarray(labels).astype(np.int64).reshape(-1)
    valid = labels != -1
    safe = np.clip(labels, 0, COLS - 1)
    rows = np.arange(labels.shape[0], dtype=np.int64)
    flat_idx = (rows * COLS + safe).astype(np.int32)
    return flat_idx, safe, valid


def kernel(**inputs):
    logits = np.ascontiguousarray(np.asarray(inputs["logits"], dtype=np.float32))
    labels = np.asarray(inputs["labels"]).reshape(-1)
    assert logits.shape == (BATCH, COLS), logits.shape
    assert labels.shape == (BATCH,), labels.shape

    from concourse.bass_utils import run_bass_kernel_spmd

    nc = _build()

    in_maps = []
    fix = []
    for c in range(N_CORES):
        r0 = c * ROWS
        flat_idx, safe, valid = _fix_arrays(labels[r0 : r0 + ROWS])
        fix.append((safe, valid))
        in_maps.append(
            {
                "logits": logits[r0 : r0 + ROWS],
                "fix_idx": flat_idx.reshape(P, RPP),
            }
        )

    global LAST_RESULTS
    LAST_RESULTS = run_bass_kernel_spmd(
        nc, in_maps, core_ids=list(range(N_CORES)), trace=TRACE
    )
    out = np.concatenate([r["out"] for r in LAST_RESULTS.results], axis=0)
    # Merge the device-computed (logit - M) * S values at each row's label.
    for c in range(N_CORES):
        safe, valid = fix[c]
        fixed = LAST_RESULTS.results[c]["fix_out"].reshape(-1)  # row p*RPP+j
        rows = np.nonzero(valid)[0]
        out[c * ROWS + rows, safe[rows]] = fixed[rows]
    return out
